# revision 9
# baseline (speedup 1.0000x reference)
"""Trainium2 Bass kernel for CentersDistance (vq_codebook).

logits[c, q] = -||centers[c] - inputs[q]||^2  for inputs [4096,128], centers [256,128].

Strategy (per spec sharding hint): shard inputs along Q across 8 cores
(512 queries/core), replicate centers. Each core computes its transposed
slab outT[q, c] = 2*dot(q,c) - ||c||^2 - ||q||^2 via TensorE matmuls:

  - inputs arrive on both HWDGE rings (sync + scalar) to overlap loads
  - PE-transpose centers (2 tiles) then x (4 tiles) into PSUM supertiles;
    ACT copies them to SBUF (x copy folds in the *2 of 2*dot)
  - qnorm: DVE square + negated free-axis reduce on the native layout
  - cnorm: DVE square of cT + GPSIMD partition_all_reduce (sum over
    partitions, broadcast to all) - no PE involvement
  - bias[p,n,c] = -qnorm[p,n] - cnorm[c]: one DVE sub of two broadcast APs
  - mains: psum[q,c] = (2*xT_n).T @ cT  (K=128, N=256), chunk-pipelined
  - epilogue per chunk: DVE add (psum_n + bias_n) -> SBUF -> DMA out on
    alternating rings, so stores overlap later chunks' compute

Host side: gather the 8 [512,256] slabs, transpose, concat -> [256,4096].
"""

import numpy as np
from contextlib import ExitStack

import concourse.bass as bass
import concourse.bacc as bacc
import concourse.tile as tile
from concourse import mybir, bass_isa
from concourse.bass_utils import run_bass_kernel_spmd
from concourse.masks import make_identity

Q, C, D = 4096, 256, 128
NCORES = 8
QL = Q // NCORES      # 512 queries per core
NQ = QL // 128        # 4 query chunks per core
NCT = C // 128        # 2 center chunks
F32 = mybir.dt.float32

_NC = None
LAST_RESULTS = None


def _build_nc():
    nc = bacc.Bacc("TRN2", target_bir_lowering=False)
    x = nc.declare_dram_parameter("x", [QL, D], F32, isOutput=False)
    cen = nc.declare_dram_parameter("c", [C, D], F32, isOutput=False)
    out = nc.declare_dram_parameter("out", [QL, C], F32, isOutput=True)

    with ExitStack() as ctx:
        tc = ctx.enter_context(tile.TileContext(nc))
        const = ctx.enter_context(tc.tile_pool(name="const", bufs=1))
        work = ctx.enter_context(tc.tile_pool(name="work", bufs=1))
        outp = ctx.enter_context(tc.tile_pool(name="outp", bufs=4))
        ptx = ctx.enter_context(
            tc.tile_pool(name="ptx", bufs=1, space=bass.MemorySpace.PSUM)
        )
        ptc = ctx.enter_context(
            tc.tile_pool(name="ptc", bufs=1, space=bass.MemorySpace.PSUM)
        )
        pm = ctx.enter_context(
            tc.tile_pool(name="pm", bufs=1, space=bass.MemorySpace.PSUM)
        )

        ident = const.tile([128, 128], F32)
        make_identity(nc, ident[:])

        # centers first (sync ring) - the c transposes lead the PE queue;
        # x halves split across both rings.
        c_raw = const.tile([128, NCT, D], F32)
        nc.sync.dma_start(c_raw[:], cen.rearrange("(n p) d -> p n d", p=128))
        x_raw = const.tile([128, NQ, D], F32)
        xr = x.rearrange("(p n) d -> p n d", n=NQ)
        nc.scalar.dma_start(x_raw[:, 0:2, :], xr[:, 0:2, :])
        nc.sync.dma_start(x_raw[:, 2:4, :], xr[:, 2:4, :])

        # centers: transpose -> cT, square -> partition all-reduce = cnorm[c]
        T_c = ptc.tile([128, NCT, 128], F32)
        for n in range(NCT):
            nc.tensor.transpose(T_c[:, n, :], c_raw[:, n, :], ident[:])
        cT = const.tile([128, C], F32)
        nc.scalar.copy(cT[:].rearrange("p (n d) -> p n d", n=NCT), T_c[:])
        cT2 = work.tile([128, C], F32)
        nc.scalar.activation(cT2[:], cT[:], mybir.ActivationFunctionType.Square)
        car = work.tile([128, 1, C], F32)
        nc.gpsimd.partition_all_reduce(
            car[:, 0, :], cT2[:], 128, bass_isa.ReduceOp.add
        )

        # -qnorm[p, n]: DVE square + negated reduce (native layout)
        x2 = work.tile([128, NQ, D], F32)
        nc.vector.tensor_mul(x2[:], x_raw[:], x_raw[:])
        qn = const.tile([128, NQ, 1], F32)
        nc.vector.tensor_reduce(
            qn[:], x2[:], mybir.AxisListType.X, mybir.AluOpType.add, negate=True
        )

        # bias[p,n,c] = -qnorm[p,n] - cnorm[c]
        bias = const.tile([128, NQ, C], F32)
        nc.vector.tensor_sub(
            bias[:],
            qn[:].broadcast_to([128, NQ, C]),
            car[:].broadcast_to([128, NQ, C]),
        )

        # chunk-pipelined: transpose -> scaled copy -> main -> epilogue -> DMA
        T_x = ptx.tile([128, NQ, 128], F32)
        xT = const.tile([128, NQ, 128], F32)
        P = pm.tile([128, NQ, 512], F32)  # 512-pad: each chunk owns a bank
        out3 = out.rearrange("(p n) c -> p n c", n=NQ)
        for n in range(NQ):
            nc.tensor.transpose(T_x[:, n, :], x_raw[:, n, :], ident[:])
            nc.scalar.mul(xT[:, n, :], T_x[:, n, :], 2.0)
            nc.tensor.matmul(P[:, n, 0:C], xT[:, n, :], cT[:], start=True, stop=True)
            o = outp.tile([128, C], F32, tag="o")
            nc.vector.tensor_add(o[:], P[:, n, 0:C], bias[:, n, :])
            eng = nc.sync if n % 2 == 0 else nc.scalar
            eng.dma_start(out3[:, n, :], o[:])

    nc.compile()  # Bacc register allocation; walrus rejects unallocated regs
    return nc


def get_nc():
    global _NC
    if _NC is None:
        _NC = _build_nc()
    return _NC


def kernel(inputs: np.ndarray, centers: np.ndarray, trace: bool = False):
    global LAST_RESULTS
    inputs = np.ascontiguousarray(np.asarray(inputs, dtype=np.float32))
    centers = np.ascontiguousarray(np.asarray(centers, dtype=np.float32))
    assert inputs.shape == (Q, D) and centers.shape == (C, D)

    nc = get_nc()
    in_maps = [
        {"x": inputs[i * QL : (i + 1) * QL], "c": centers} for i in range(NCORES)
    ]
    res = run_bass_kernel_spmd(nc, in_maps, list(range(NCORES)), trace=trace)
    LAST_RESULTS = res
    full = np.empty((C, Q), dtype=np.float32)
    for i in range(NCORES):
        full[:, i * QL : (i + 1) * QL] = res.results[i]["out"].T
    return full


# revision 10
# speedup vs baseline: 1.1301x; 1.1301x over previous
"""Trainium2 Bass kernel for CentersDistance (vq_codebook).

logits[c, q] = -||centers[c] - inputs[q]||^2  for inputs [4096,128], centers [256,128].

Sharding (per spec hint): shard inputs along Q across 8 cores (512
queries/core), replicate centers; each core computes its [C, 512] slab
independently, no collectives.

The kernel() wrapper prepares per-core layouts on the host (sharding-time
layout prep; ~0.2% of the FLOPs - the O(C*Q*D) distance matmul stays on
device): D-major transposed shards xT [128,512] / cT [128,256], plus the
negated norm vectors -||x_q||^2 (row) and -||c_c||^2 (per-partition column).

Per-core device graph (logits = 2*x.c - ||x||^2 - ||c||^2):
  - DMA in: xT (sync ring) | cT, -qnorm row, -cnorm col (scalar ring)
  - GPSIMD partition_broadcast replicates the -qnorm row to 128 partitions
  - DVE: bias[p,h,q] = -cnorm[128,2,1] + -qnorm[128,512] (broadcast APs)
  - PE: 2 matmuls psum[h] = (2*cT_h).T @ xT  (K=128, M=128, N=512, fp32;
    the *2 is folded into cT on the host)
  - DVE: out_h = psum_h + bias_h -> SBUF; 2 output DMAs on both rings
"""

import numpy as np
from contextlib import ExitStack

import concourse.bass as bass
import concourse.bacc as bacc
import concourse.tile as tile
from concourse import mybir
from concourse.bass_utils import run_bass_kernel_spmd

Q, C, D = 4096, 256, 128
NCORES = 8
QL = Q // NCORES      # 512 queries per core
NCT = C // 128        # 2 center chunks
F32 = mybir.dt.float32

_NC = None
LAST_RESULTS = None


def _build_nc():
    nc = bacc.Bacc("TRN2", target_bir_lowering=False)
    xT = nc.declare_dram_parameter("xT", [D, QL], F32, isOutput=False)
    cT = nc.declare_dram_parameter("cT", [D, C], F32, isOutput=False)
    nqn = nc.declare_dram_parameter("nqn", [1, QL], F32, isOutput=False)
    ncn = nc.declare_dram_parameter("ncn", [128, NCT, 1], F32, isOutput=False)
    out = nc.declare_dram_parameter("out", [C, QL], F32, isOutput=True)

    with ExitStack() as ctx:
        tc = ctx.enter_context(tile.TileContext(nc))
        const = ctx.enter_context(tc.tile_pool(name="const", bufs=1))
        outp = ctx.enter_context(tc.tile_pool(name="outp", bufs=2))
        pm = ctx.enter_context(
            tc.tile_pool(name="pm", bufs=1, space=bass.MemorySpace.PSUM)
        )

        xT_sb = const.tile([D, QL], F32)
        nc.sync.dma_start(xT_sb[:], xT[:, :])
        cT_sb = const.tile([D, C], F32)
        nc.scalar.dma_start(cT_sb[:], cT[:, :])
        nqn_sb = const.tile([1, QL], F32)
        nc.scalar.dma_start(nqn_sb[:], nqn[:, :])
        ncn_sb = const.tile([128, NCT, 1], F32)
        nc.scalar.dma_start(ncn_sb[:], ncn[:, :, :])

        # replicate -qnorm row across partitions, then fuse with -cnorm col
        nqb = const.tile([128, QL], F32)
        nc.gpsimd.partition_broadcast(nqb[:], nqn_sb[:])
        bias = const.tile([128, NCT, QL], F32)
        nc.vector.tensor_add(
            bias[:],
            ncn_sb[:].broadcast_to([128, NCT, QL]),
            nqb[:].rearrange("p (o q) -> p o q", o=1).broadcast_to([128, NCT, QL]),
        )

        # mains + epilogue, chunked over the 2 center blocks
        P = pm.tile([128, NCT, QL], F32)  # one full PSUM bank per chunk
        for h in range(NCT):
            nc.tensor.matmul(
                P[:, h, :], cT_sb[:, bass.ts(h, 128)], xT_sb[:], start=True, stop=True
            )
            o = outp.tile([128, QL], F32, tag="o")
            nc.vector.tensor_add(o[:], P[:, h, :], bias[:, h, :])
            eng = nc.sync if h == 0 else nc.scalar
            eng.dma_start(out[bass.ts(h, 128), :], o[:])

    nc.compile()  # Bacc register allocation; walrus rejects unallocated regs
    return nc


def get_nc():
    global _NC
    if _NC is None:
        _NC = _build_nc()
    return _NC


def kernel(inputs: np.ndarray, centers: np.ndarray, trace: bool = False):
    global LAST_RESULTS
    inputs = np.asarray(inputs, dtype=np.float32)
    centers = np.asarray(centers, dtype=np.float32)
    assert inputs.shape == (Q, D) and centers.shape == (C, D)

    # host-side layout prep: transposed shards + negated norms
    cT2x = np.ascontiguousarray(2.0 * centers.T)                      # [D, C]
    ncn = np.ascontiguousarray(
        -(centers.astype(np.float64) ** 2).sum(1).astype(np.float32)
        .reshape(NCT, 128).T.reshape(128, NCT, 1)                     # [128,NCT,1]
    )
    nc_ = get_nc()
    in_maps = []
    for i in range(NCORES):
        xs = inputs[i * QL : (i + 1) * QL]
        in_maps.append({
            "xT": np.ascontiguousarray(xs.T),
            "cT": cT2x,
            "nqn": np.ascontiguousarray(
                -(xs.astype(np.float64) ** 2).sum(1).astype(np.float32)[None, :]
            ),
            "ncn": ncn,
        })
    res = run_bass_kernel_spmd(nc_, in_maps, list(range(NCORES)), trace=trace)
    LAST_RESULTS = res
    full = np.empty((C, Q), dtype=np.float32)
    for i in range(NCORES):
        full[:, i * QL : (i + 1) * QL] = res.results[i]["out"]
    return full


# revision 13
# speedup vs baseline: 1.4991x; 1.3265x over previous
"""Trainium2 Bass kernel for CentersDistance (vq_codebook).

logits[c, q] = -||centers[c] - inputs[q]||^2  for inputs [4096,128], centers [256,128].

Sharding (per spec hint): shard inputs along Q across 8 cores (512
queries/core), replicate centers; each core computes its [C, 512] slab
independently, no collectives.

kernel() prepares per-core layouts on the host (sharding-time layout prep;
~0.2% of FLOPs - the O(C*Q*D) distance matmul stays on device): D-major
transposed shards xT [128,512] and 2*cT [128,256], the negated query norms
as a per-partition column [128,4,1], and -cnorm replicated to [128,256].

Per-core device graph (logits = 2*x.c - ||x||^2 - ||c||^2, computed as
outT[q, c] then transposed on gather):
  - DMA in: xT + qnorm col (sync ring) | 2cT + cnorm replica (scalar ring)
  - DVE: bias[p,n,c] = -qnorm[p,n] + -cnorm[c] (two broadcast APs, one add)
  - per q-chunk n (4): PE matmul psum_n[q,c] = xT_n.T @ (2cT) (K=128,N=256)
    into its own PSUM bank; DVE add psum_n + bias_n -> SBUF; DMA out on
    alternating rings so stores overlap later chunks' compute

Host gathers the 8 [512,256] slabs, transposes, concats -> [256,4096].
"""

import numpy as np
from contextlib import ExitStack

import concourse.bass as bass
import concourse.bacc as bacc
import concourse.tile as tile
from concourse import mybir
from concourse.bass_utils import run_bass_kernel_spmd

Q, C, D = 4096, 256, 128
NCORES = 8
QL = Q // NCORES      # 512 queries per core
NQ = QL // 128        # 4 query chunks per core
F32 = mybir.dt.float32

_NC = None
LAST_RESULTS = None


def _build_nc():
    nc = bacc.Bacc("TRN2", target_bir_lowering=False)
    xT = nc.declare_dram_parameter("xT", [D, QL], F32, isOutput=False)
    cT = nc.declare_dram_parameter("cT", [D, C], F32, isOutput=False)
    nqn = nc.declare_dram_parameter("nqn", [128, NQ, 1], F32, isOutput=False)
    ncr = nc.declare_dram_parameter("ncr", [128, C], F32, isOutput=False)
    out = nc.declare_dram_parameter("out", [QL, C], F32, isOutput=True)

    with ExitStack() as ctx:
        tc = ctx.enter_context(tile.TileContext(nc))
        const = ctx.enter_context(tc.tile_pool(name="const", bufs=1))
        outp = ctx.enter_context(tc.tile_pool(name="outp", bufs=4))
        pm = ctx.enter_context(
            tc.tile_pool(name="pm", bufs=4, space=bass.MemorySpace.PSUM)
        )

        cT_sb = const.tile([D, C], F32)
        nc.scalar.dma_start(cT_sb[:], cT[:, :])
        xT_sb = const.tile([D, QL], F32)
        nc.sync.dma_start(xT_sb[:], xT[:, :])
        ncr_sb = const.tile([128, 1, C], F32)
        nc.scalar.dma_start(ncr_sb[:, 0, :], ncr[:, :])
        nqn_sb = const.tile([128, NQ, 1], F32)
        nc.sync.dma_start(nqn_sb[:], nqn[:, :, :])

        # bias[p,n,c] = -qnorm[p,n] + -cnorm[c]
        bias = const.tile([128, NQ, C], F32)
        nc.vector.tensor_add(
            bias[:],
            nqn_sb[:].broadcast_to([128, NQ, C]),
            ncr_sb[:].broadcast_to([128, NQ, C]),
        )

        # chunk-pipelined mains + epilogue; each chunk owns a PSUM bank
        for n in range(NQ):
            ps = pm.tile([128, C], F32, tag="ps")
            nc.tensor.matmul(
                ps[:], xT_sb[:, bass.ts(n, 128)], cT_sb[:], start=True, stop=True
            )
            o = outp.tile([128, C], F32, tag="o")
            nc.vector.tensor_add(o[:], ps[:], bias[:, n, :])
            eng = nc.sync if n % 2 == 0 else nc.scalar
            eng.dma_start(out[bass.ts(n, 128), :], o[:])

    nc.compile()  # Bacc register allocation; walrus rejects unallocated regs
    return nc


def get_nc():
    global _NC
    if _NC is None:
        _NC = _build_nc()
    return _NC


def kernel(inputs: np.ndarray, centers: np.ndarray, trace: bool = False):
    global LAST_RESULTS
    inputs = np.asarray(inputs, dtype=np.float32)
    centers = np.asarray(centers, dtype=np.float32)
    assert inputs.shape == (Q, D) and centers.shape == (C, D)

    # host-side layout prep: transposed shards + negated norms
    cT2x = np.ascontiguousarray(2.0 * centers.T)                      # [D, C]
    ncn = -(centers.astype(np.float64) ** 2).sum(1).astype(np.float32)
    ncr = np.ascontiguousarray(np.broadcast_to(ncn[None, :], (128, C)))
    nc_ = get_nc()
    in_maps = []
    for i in range(NCORES):
        xs = inputs[i * QL : (i + 1) * QL]
        nq = -(xs.astype(np.float64) ** 2).sum(1).astype(np.float32)
        in_maps.append({
            "xT": np.ascontiguousarray(xs.T),
            "cT": cT2x,
            # chunk n covers queries n*128..n*128+127; partition p = offset
            "nqn": np.ascontiguousarray(nq.reshape(NQ, 128).T.reshape(128, NQ, 1)),
            "ncr": ncr,
        })
    res = run_bass_kernel_spmd(nc_, in_maps, list(range(NCORES)), trace=trace)
    LAST_RESULTS = res
    full = np.empty((C, Q), dtype=np.float32)
    for i in range(NCORES):
        full[:, i * QL : (i + 1) * QL] = res.results[i]["out"].T
    return full
